# revision 5
# baseline (speedup 1.0000x reference)
"""Multi-head attention (B=2, S=2048, D=1024, H=16) on 8 TRN2 NeuronCores.

Sharding: core c -> (batch b = c//4, head-group g = c%4 of 4 heads / 256 dims).
Per core: QKV projections for its head slice, attention for its 4 heads,
softmax normalization, AllGather of attention outputs across the 4 cores of
the batch group, then the core's 256-column slice of the output projection.
Host side only transposes/casts/slices inputs and concatenates outputs.

Layout notes:
- Activations are kept transposed ([feature, seq]) so every matmul contracts
  on the partition axis without on-chip transposes.
- Scores are computed transposed ([kseq, q]); softmax row sums come from 64
  ones-columns appended to each head of V, so the PV matmul emits the row sum
  replicated across partitions 64..127 and normalization is plain DVE math.
- No max-subtraction in softmax: scores are ~N(0,1) after the 1/sqrt(dk)
  scale (|s| < ~7 over 134M samples), safely inside exp's fp32 range.
"""

import numpy as np
import ml_dtypes

import concourse.bass as bass
import concourse.mybir as mybir
import concourse.tile as tile
from concourse.bass_utils import run_bass_kernel_spmd
from concourse.vector_clock import ScopedClock

BF16 = ml_dtypes.bfloat16
F32 = mybir.dt.float32
BF = mybir.dt.bfloat16

B, S, D, H = 2, 2048, 1024, 16
DK = D // H          # 64
HPC = H // 4         # 4 heads per core
EG = D // 4          # 256 dims per head-group
KT = D // 128        # 8 contraction tiles
GROUPS = [[0, 1, 2, 3], [4, 5, 6, 7]]
EXP = mybir.ActivationFunctionType.Exp

TRACE = False
LAST_EXEC_NS = None


# --- workaround: this walrus build only encodes ONE sync wait per
# instruction ("Too many sync wait commands" in setupSyncWait). Hoist
# excess waits onto same-engine NOP carriers placed just before the
# instruction; engines execute in order, so semantics are unchanged. ---
def _split_multi_waits(nc, max_waits=1):
    n = 0
    for f in nc.m.functions:
        for bb in f.blocks:
            new = []
            for inst in bb.instructions:
                si = inst.sync_info
                waits = list(si.on_wait) if si is not None and si.on_wait else []
                if len(waits) > max_waits:
                    keep = len(waits) - max_waits
                    for j in range(0, keep, max_waits):
                        n += 1
                        new.append(
                            mybir.InstNoOp(
                                name=f"waitsplit-{n}",
                                engine=inst.engine,
                                bass_nofuse=True,
                                sync_info=mybir.SyncInfo(
                                    on_wait=waits[j : j + max_waits], on_update=[]
                                ),
                            )
                        )
                    si.on_wait = waits[keep:]
                new.append(inst)
            bb.instructions[:] = new
    return n


def build(s=S):
    """Build the per-core SPMD program. s = sequence length (tunable for sim)."""
    n_sc = s // 512   # 512-wide q chunks
    n_st = s // 128   # 128-wide seq tiles

    nc = bass.Bass(num_devices=8)
    xq_t = nc.declare_dram_parameter("xq_t", [D, s], BF, isOutput=False)
    xk_t = nc.declare_dram_parameter("xk_t", [D, s], BF, isOutput=False)
    xv_t = nc.declare_dram_parameter("xv_t", [D, s], BF, isOutput=False)
    wq_t = nc.declare_dram_parameter("wq_t", [D, EG], BF, isOutput=False)
    wk_t = nc.declare_dram_parameter("wk_t", [D, EG], BF, isOutput=False)
    wv_t = nc.declare_dram_parameter("wv_t", [D, EG], BF, isOutput=False)
    wo_t = nc.declare_dram_parameter("wo_t", [D, EG], BF, isOutput=False)
    y_ext = nc.declare_dram_parameter("y", [s, EG], F32, isOutput=True)

    attn_bounce = nc.dram_tensor("attn_bounce", [EG, s], BF)
    attn_gath = nc.dram_tensor("attn_gath", [D, s], BF)

    with tile.TileContext(nc) as tc:
        with (
            tc.tile_pool(name="persist", bufs=1) as pp,
            tc.tile_pool(name="wo_pool", bufs=1) as wop,
        ):
            # persistent SBUF tensors
            qT = [pp.tile([128, s], BF, tag=f"qT{e}", name=f"qT{e}") for e in range(2)]
            kT = [pp.tile([128, s], BF, tag=f"kT{e}", name=f"kT{e}") for e in range(2)]
            vE = [pp.tile([128, HPC * 2 * DK], BF, tag=f"vE{t}", name=f"vE{t}") for t in range(n_st)]
            attn_sb = [pp.tile([128, s], BF, tag=f"attn{t}", name=f"attn{t}") for t in range(2)]
            wo_sb = [wop.tile([128, EG], BF, tag=f"wo{k}", name=f"wo{k}") for k in range(KT)]
            for k in range(KT):
                nc.sync.dma_start(wo_sb[k][:], wo_t[k * 128:(k + 1) * 128, :])

            # ---------------- phase 1: projections ----------------
            with (
                tc.tile_pool(name="xin", bufs=1) as xp,
                tc.tile_pool(name="win", bufs=1) as wp,
                tc.tile_pool(name="psum1", bufs=2, space="PSUM") as ps1,
            ):
                xq = [xp.tile([128, s], BF, tag=f"xq{k}", name=f"xq{k}") for k in range(KT)]
                xk = [xp.tile([128, s], BF, tag=f"xk{k}", name=f"xk{k}") for k in range(KT)]
                xv = [xp.tile([128, s], BF, tag=f"xv{k}", name=f"xv{k}") for k in range(KT)]
                wq = [wp.tile([128, EG], BF, tag=f"wq{k}", name=f"wq{k}") for k in range(KT)]
                wk = [wp.tile([128, EG], BF, tag=f"wk{k}", name=f"wk{k}") for k in range(KT)]
                wv = [wp.tile([128, EG], BF, tag=f"wv{k}", name=f"wv{k}") for k in range(KT)]
                for k in range(KT):
                    sl = slice(k * 128, (k + 1) * 128)
                    nc.sync.dma_start(xq[k][:], xq_t[sl, :])
                    nc.sync.dma_start(xk[k][:], xk_t[sl, :])
                    nc.sync.dma_start(xv[k][:], xv_t[sl, :])
                    nc.sync.dma_start(wq[k][:], wq_t[sl, :])
                    nc.sync.dma_start(wk[k][:], wk_t[sl, :])
                    nc.sync.dma_start(wv[k][:], wv_t[sl, :])

                # q/k projections, transposed layout [e, s]
                for w_sb, x_sb, dst in ((wq, xq, qT), (wk, xk, kT)):
                    for c in range(n_sc):
                        cs = slice(c * 512, (c + 1) * 512)
                        for e in range(2):
                            ps = ps1.tile([128, 512], F32, tag="proj_qk")
                            for k in range(KT):
                                nc.tensor.matmul(
                                    ps[:],
                                    w_sb[k][:, e * 128:(e + 1) * 128],
                                    x_sb[k][:, cs],
                                    start=(k == 0),
                                    stop=(k == KT - 1),
                                )
                            nc.vector.tensor_copy(dst[e][:, cs], ps[:])

                # v projection, natural layout [s, e], + ones column per head
                for t in range(n_st):
                    ts_ = slice(t * 128, (t + 1) * 128)
                    ps = ps1.tile([128, EG], F32, tag="proj_v")
                    for k in range(KT):
                        nc.tensor.matmul(
                            ps[:],
                            xv[k][:, ts_],
                            wv[k][:],
                            start=(k == 0),
                            stop=(k == KT - 1),
                        )
                    nc.vector.memset(vE[t][:], 1.0)
                    for h in range(HPC):
                        nc.vector.tensor_copy(
                            vE[t][:, h * 2 * DK:h * 2 * DK + DK],
                            ps[:, h * DK:(h + 1) * DK],
                        )

            # ---------------- phase 2: attention ----------------
            with (
                tc.tile_pool(name="psum2", bufs=1, space="PSUM") as ps2,
                tc.tile_pool(name="expp", bufs=3) as ep,
                tc.tile_pool(name="normp", bufs=2) as np_,
            ):
                for c in range(n_sc):
                    cs = slice(c * 512, (c + 1) * 512)
                    attnP = [
                        ps2.tile([2 * DK, 512], F32, tag=f"attnP{h}", name=f"attnP{h}")
                        for h in range(HPC)
                    ]
                    for t in range(n_st):
                        ts_ = slice(t * 128, (t + 1) * 128)
                        for pair in range(2):
                            scp = ps2.tile([128, 1024], F32, tag=f"scores{pair}")
                            for sub in range(2):
                                row = slice(64 * sub, 64 * sub + 64)
                                nc.tensor.matmul(
                                    scp[:, sub * 512:(sub + 1) * 512],
                                    kT[pair][row, ts_],
                                    qT[pair][row, cs],
                                    start=True,
                                    stop=True,
                                )
                            ex = ep.tile([128, 1024], BF, tag=f"expT{pair}")
                            nc.scalar.activation(ex[:], scp[:], EXP, scale=1.0 / 8.0)
                            for sub in range(2):
                                h = 2 * pair + sub
                                nc.tensor.matmul(
                                    attnP[h][:],
                                    vE[t][:, h * 2 * DK:(h + 1) * 2 * DK],
                                    ex[:, sub * 512:(sub + 1) * 512],
                                    start=(t == 0),
                                    stop=(t == n_st - 1),
                                )
                    for h in range(HPC):
                        den = np_.tile([DK, 512], F32, tag="den")
                        nc.vector.reciprocal(den[:], attnP[h][DK:2 * DK, :])
                        nc.vector.tensor_mul(
                            attn_sb[h // 2][64 * (h % 2):64 * (h % 2) + 64, cs],
                            attnP[h][0:DK, :],
                            den[:],
                        )

            # ---------------- phase 3: AllGather + out-projection ----------------
            for t in range(2):
                nc.sync.dma_start(attn_bounce[t * 128:(t + 1) * 128, :], attn_sb[t][:])
            nc.gpsimd.collective_compute(
                "AllGather",
                mybir.AluOpType.bypass,
                replica_groups=GROUPS,
                ins=[attn_bounce[:]],
                outs=[attn_gath[:]],
            )
            with (
                tc.tile_pool(name="agp", bufs=1) as agp,
                tc.tile_pool(name="yp", bufs=3) as yp,
                tc.tile_pool(name="psum3", bufs=2, space="PSUM") as ps3,
            ):
                ag = [agp.tile([128, s], BF, tag=f"ag{k}", name=f"ag{k}") for k in range(KT)]
                for k in range(KT):
                    nc.sync.dma_start(ag[k][:], attn_gath[k * 128:(k + 1) * 128, :])
                for t in range(n_st):
                    ts_ = slice(t * 128, (t + 1) * 128)
                    ps = ps3.tile([128, EG], F32, tag="yps")
                    for k in range(KT):
                        nc.tensor.matmul(
                            ps[:],
                            ag[k][:, ts_],
                            wo_sb[k][:],
                            start=(k == 0),
                            stop=(k == KT - 1),
                        )
                    ysb = yp.tile([128, EG], F32, tag="ysb")
                    nc.vector.tensor_copy(ysb[:], ps[:])
                    nc.sync.dma_start(y_ext[ts_, :], ysb[:])

    _split_multi_waits(nc)
    return nc


def _bf16_c(a):
    return np.ascontiguousarray(a).astype(BF16)


def kernel(query, key, value, Wq, bq, Wk, bk, Wv, bv, Wo, bo):
    global LAST_EXEC_NS
    query, key, value = (np.asarray(a, np.float32) for a in (query, key, value))
    Wq, Wk, Wv, Wo = (np.asarray(a, np.float32) for a in (Wq, Wk, Wv, Wo))
    for b_ in (bq, bk, bv, bo):
        assert not np.any(np.asarray(b_)), "nonzero biases not supported"

    nc = build(S)
    in_maps = []
    for c in range(8):
        b, g = divmod(c, 4)
        eg = slice(EG * g, EG * (g + 1))
        in_maps.append(
            {
                "xq_t": _bf16_c(query[b].T),
                "xk_t": _bf16_c(key[b].T),
                "xv_t": _bf16_c(value[b].T),
                "wq_t": _bf16_c(Wq[eg].T),
                "wk_t": _bf16_c(Wk[eg].T),
                "wv_t": _bf16_c(Wv[eg].T),
                "wo_t": _bf16_c(Wo[eg].T),
            }
        )
    res = run_bass_kernel_spmd(nc, in_maps, list(range(8)), trace=TRACE)
    LAST_EXEC_NS = res.exec_time_ns
    y = np.empty((B, S, D), np.float32)
    for c in range(8):
        b, g = divmod(c, 4)
        y[b][:, EG * g:EG * (g + 1)] = res.results[c]["y"]
    return y
